# revision 2
# baseline (speedup 1.0000x reference)
"""EventWarping (contrast-maximization loss) Trainium2 kernel, v2.

v2 = polarity-sorted sharding: the host partitions each batch's events
by polarity, so every 128-event chunk on a core is single-polarity.
This removes the polarity masking entirely and moves the ts-weighting
to the moving operand:

    rhs  = [ -gx | -gx*ts ]   [128, 512]  (column indicators, negated)
    lhsT = -gy                [128, 256]  (row indicator, negated)
    bank[pass*2+h] += (-gy)[:, h*128:].T @ rhs     (products positive)

2 matmuls per (chunk, pass) instead of the baseline's 4.  The bilinear
triangle of side s at distance d = iota - w is accumulated negated:
-relu(1 - |d|) = min(|d| - 1, 0), built either as

    ACT path:  abs = Abs(iota + (-w))            (1 ACT op, AP bias)
    DVE path:  a = (iota - w) - 1                (TS)
               m = ((-iota-1) + w) max a         (STT) = |d| - 1
    then       min(. - 1 or . , 0)               (TS, DVE or GPSIMD)

Engine assignment per (chunk, pass) is policy-driven (env-tunable) to
balance DVE / ACT / GPSIMD.  All 4 accumulating histograms (2 passes x
2 row-halves, each [iwe | ts_iwe]) live in 4 PSUM banks for the whole
kernel.

Sharding: core = batch*4 + pol*2 + half; each core takes ~half of its
(batch, polarity) events (~250k), padded with x=-10 events (zero
contribution).  Per-core partial histograms are summed and the (tiny)
normalization/loss reduction computed on the host after gathering.
"""

import os

import numpy as np

import concourse.bacc as bacc
import concourse.bass as bass
import concourse.mybir as mybir
import concourse.tile as tile
from concourse.bass_utils import run_bass_kernel_spmd

P = 128
HW = 256          # histogram height/width
GS = 32           # chunks per group (one For_i iteration)
NG = 62           # groups per core
NCH = NG * GS     # 1984 chunks/core
NCORES = 8
FS = 256.0        # flow scaling
EPS = 1e-9
PAD_X = -10.0     # pad events: |iota - (-10)| >= 9 -> zero triangle

F16 = mybir.dt.float16
F32 = mybir.dt.float32
AF = mybir.ActivationFunctionType
OP = mybir.AluOpType

LAST_EXEC_NS = None
LAST_RESULTS = None


def build_program(ng=NG, loop_ng=None):
    """Builds the SPMD single-core program (identical on all 8 cores).
    loop_ng: process only the first loop_ng groups (same I/O shapes) —
    used to measure pure loop time by differencing two builds."""
    if loop_ng is None:
        loop_ng = ng
    nc = bacc.Bacc("TRN2", target_bir_lowering=False, debug=False,
                   num_devices=NCORES)

    fields = nc.dram_tensor("fields", [P, ng * 5 * GS], F32,
                            kind="ExternalInput")
    iotas = nc.dram_tensor("iotas", [P, 2 * HW], F16, kind="ExternalInput")
    hist = nc.dram_tensor("hist", [4, P, 512], F32, kind="ExternalOutput")

    # --- engine policy (per chunk index c, per side) -------------------
    # abs builder: 'a' = ACT Abs (1 op), 'd' = DVE a+m (2 ops)
    # min op owner: 'g' = GPSIMD, 'd' = DVE
    # rhs2 owner: 'd' = DVE, 'a' = ACT (Copy w/ scale)
    # Defaults balance DVE/ACT/GPS per the cost model.
    px = os.environ.get("KPX", "aa")     # absx path, cycled over chunks
    py = os.environ.get("KPY", "da")     # absy path, cycled
    pny = os.environ.get("KPNY", "gg")   # negy min-op owner, cycled
    pr2 = os.environ.get("KPR2", "dd")   # rhs2 owner, cycled

    with tile.TileContext(nc) as tc:
        with (
            tc.tile_pool(name="const", bufs=1) as constp,
            tc.tile_pool(name="stage", bufs=2) as stagep,
            tc.tile_pool(name="drv", bufs=2) as drvp,
            tc.tile_pool(name="oh", bufs=int(os.environ.get("KBUFS", "4"))) as ohp,
            tc.tile_pool(name="rhs", bufs=int(os.environ.get("KBUFS", "4"))) as rhsp,
            tc.tile_pool(name="psum", bufs=1, space="PSUM") as psump,
            tc.tile_pool(name="out", bufs=1) as outp,
        ):
            iot = constp.tile([P, 2 * HW], F16)
            nc.sync.dma_start(iot[:], iotas.ap())
            iota_c = iot[:, 0:HW]            # c (0..255)
            niota_m1 = iot[:, HW:2 * HW]     # -c - 1

            zl = constp.tile([P, P], F16)
            nc.vector.memset(zl[:], 0.0)
            zr = constp.tile([P, 512], F16)
            nc.vector.memset(zr[:], 0.0)

            # 4 accumulator banks: [pass(2) x half(2)] x [128, 512];
            # columns = [iwe(256) | ts_iwe(256)]
            banks = [psump.tile([P, 512], F32, tag=f"bank{i}",
                                name=f"bank{i}")
                     for i in range(4)]
            for b in banks:
                nc.tensor.matmul(b[:], zl[:], zr[:], start=True, stop=False)

            hints_s = os.environ.get("KHINT", "pd")
            hmap = {"p": (mybir.EngineType.PE,),
                    "pd": (mybir.EngineType.PE, mybir.EngineType.DVE),
                    "none": ()}
            hints = hmap.get(hints_s, hmap["pd"])
            stag = os.environ.get("KSTAG", "0") == "1"
            with tc.For_i(0, loop_ng * 5 * GS, 5 * GS,
                          hint_engines=hints, staggered_reset=stag) as g0:
                st = stagep.tile([P, 5 * GS], F32)
                nc.sync.dma_start(st[:], fields.ap()[:, bass.ds(g0, 5 * GS)])
                ts_ = st[:, 0 * GS:1 * GS]
                x_ = st[:, 1 * GS:2 * GS]
                y_ = st[:, 2 * GS:3 * GS]
                fx_ = st[:, 3 * GS:4 * GS]
                fy_ = st[:, 4 * GS:5 * GS]

                # ---- per-group derived quantities [P, GS] (fp32) ----
                need_w = {"x": "d" in px, "y": "d" in py}
                need_nw = {"x": "a" in px, "y": "a" in py}
                d = {}

                def drv(k):
                    if k not in d:
                        d[k] = drvp.tile([P, GS], F32, tag=k, name=k)
                    return d[k]

                nc.vector.tensor_tensor(drv("g1")[:], fx_, ts_, OP.mult)
                nc.vector.tensor_tensor(drv("g2")[:], fy_, ts_, OP.mult)
                # w_b = pos - 256*g ; w_f = w_b + 256*f
                nc.vector.scalar_tensor_tensor(drv("wxb")[:], drv("g1")[:],
                                               -FS, x_, OP.mult, OP.add)
                nc.vector.scalar_tensor_tensor(drv("wxf")[:], fx_, FS,
                                               drv("wxb")[:], OP.mult, OP.add)
                nc.vector.scalar_tensor_tensor(drv("wyb")[:], drv("g2")[:],
                                               -FS, y_, OP.mult, OP.add)
                nc.vector.scalar_tensor_tensor(drv("wyf")[:], fy_, FS,
                                               drv("wyb")[:], OP.mult, OP.add)
                # negated forms for the ACT Abs bias
                if need_nw["x"]:
                    nc.vector.tensor_scalar(drv("nwxb")[:], drv("wxb")[:],
                                            -1.0, None, OP.mult)
                    nc.vector.tensor_scalar(drv("nwxf")[:], drv("wxf")[:],
                                            -1.0, None, OP.mult)
                if need_nw["y"]:
                    nc.vector.tensor_scalar(drv("nwyb")[:], drv("wyb")[:],
                                            -1.0, None, OP.mult)
                    nc.vector.tensor_scalar(drv("nwyf")[:], drv("wyf")[:],
                                            -1.0, None, OP.mult)

                def build_abs(path, w, nw, c, tag):
                    """Returns (tile, kind): kind 'abs' => |d|, 'absm1'
                    => |d|-1 (negated-triangle precursor)."""
                    if path == "a":
                        t = ohp.tile([P, HW], F16, tag=tag)
                        nc.scalar.activation(t[:], iota_c, AF.Abs,
                                             bias=nw[:, c:c + 1], scale=1.0)
                        return t, "abs"
                    a = ohp.tile([P, HW], F16, tag=tag + "a")
                    nc.vector.tensor_scalar(a[:], iota_c, w[:, c:c + 1],
                                            1.0, OP.subtract, OP.subtract)
                    m = ohp.tile([P, HW], F16, tag=tag + "m")
                    nc.vector.scalar_tensor_tensor(m[:], niota_m1,
                                                   w[:, c:c + 1], a[:],
                                                   OP.add, OP.max)
                    return m, "absm1"

                def min0(eng, out, t, kind):
                    """out = min(t - (1 if kind=='abs' else 0), 0)"""
                    sub = 1.0 if kind == "abs" else 0.0
                    e = nc.gpsimd if eng == "g" else nc.vector
                    if sub:
                        e.tensor_scalar(out, t[:], 1.0, 0.0,
                                        OP.subtract, OP.min)
                    else:
                        e.tensor_scalar(out, t[:], 0.0, None, OP.min)

                passes = ((0, "wxf", "nwxf", "wyf", "nwyf"),
                          (1, "wxb", "nwxb", "wyb", "nwyb"))
                for c in range(GS):
                    cx = px[c % len(px)]
                    cy = py[c % len(py)]
                    cny = pny[c % len(pny)]
                    cr2 = pr2[c % len(pr2)]
                    for (pi, wxk, nwxk, wyk, nwyk) in passes:
                        wx = d.get(wxk)
                        nwx = d.get(nwxk)
                        wy = d.get(wyk)
                        nwy = d.get(nwyk)
                        # column indicator (negated) -> rhs[:, 0:256]
                        ax, kx = build_abs(cx, wx, nwx, c, "absx")
                        rhs = rhsp.tile([P, 512], F16, tag="rhs")
                        min0("d", rhs[:, 0:HW], ax, kx)
                        if cr2 == "a":
                            nc.scalar.mul(rhs[:, HW:512], rhs[:, 0:HW],
                                          ts_[:, c:c + 1])
                        else:
                            nc.vector.tensor_scalar(rhs[:, HW:512],
                                                    rhs[:, 0:HW],
                                                    ts_[:, c:c + 1], None,
                                                    OP.mult)
                        # row indicator (negated), stationary
                        ay, ky = build_abs(cy, wy, nwy, c, "absy")
                        negy = ohp.tile([P, HW], F16, tag="negy")
                        min0(cny, negy[:], ay, ky)
                        for h in (0, 1):
                            nc.tensor.matmul(banks[pi * 2 + h][:],
                                             negy[:, h * P:(h + 1) * P],
                                             rhs[:], start=False, stop=False)

            # close accumulation groups
            for b in banks:
                nc.tensor.matmul(b[:], zl[:], zr[:], start=False, stop=True)
            # drain PSUM -> SBUF -> DRAM
            for i, b in enumerate(banks):
                ob = outp.tile([P, 512], F32, tag=f"ob{i}")
                if i % 2 == 0:
                    nc.vector.tensor_copy(ob[:], b[:])
                else:
                    nc.scalar.copy(ob[:], b[:])
                nc.sync.dma_start(hist.ap()[i], ob[:])

    nc.compile()
    return nc


def _iota_arrays():
    c = np.arange(HW, dtype=np.float32)
    rows = np.concatenate([c, -c - 1.0]).astype(np.float16)
    return np.broadcast_to(rows, (P, 2 * HW)).copy()


def _pack_fields(ev, fl, ng=NG):
    """ev [n,4] fp32, fl [n,2] fp32 -> [P, ng*5*GS] fp32 (x=PAD_X pad).
    Field order per group: ts, x, y, fx, fy."""
    nch = ng * GS
    evc = nch * P
    n = ev.shape[0]
    assert n <= evc, (n, evc)
    arr = np.zeros((5, evc), dtype=np.float32)
    arr[1, :] = PAD_X
    arr[0, :n] = ev[:, 0]
    arr[1, :n] = ev[:, 1]
    arr[2, :n] = ev[:, 2]
    arr[3, :n] = fl[:, 0]
    arr[4, :n] = fl[:, 1]
    # [5, nch, P] -> [P, ng, 5, GS]
    a = arr.reshape(5, ng, GS, P)
    return np.ascontiguousarray(a.transpose(3, 1, 0, 2)).reshape(P, ng * 5 * GS)


_PROG = {}


def _get_prog():
    if "nc" not in _PROG:
        _PROG["nc"] = build_program(NG)
    return _PROG["nc"]


def loss_from_hists(hists):
    """hists: [2 batches][2 pols] arrays [4,128,512] (summed over the
    cores of that shard). Returns the scalar loss (float64)."""
    total = 0.0
    for b in range(2):
        for pi in range(2):            # pass: 0=fw, 1=bw
            num = 0.0
            iwe_sum = np.zeros((2 * P, HW), np.float64)
            for pol in range(2):
                hb = hists[b][pol]
                iwe = np.concatenate([hb[pi * 2 + 0][:, 0:HW],
                                      hb[pi * 2 + 1][:, 0:HW]], axis=0)
                tsw = np.concatenate([hb[pi * 2 + 0][:, HW:512],
                                      hb[pi * 2 + 1][:, HW:512]], axis=0)
                num += ((tsw / (iwe + EPS)) ** 2).sum()
                iwe_sum += iwe
            nz = (iwe_sum > 0).sum()
            total += num / nz
    return total


def make_in_maps(events, flow):
    events = np.asarray(events, dtype=np.float32)
    flow = np.asarray(flow, dtype=np.float32)
    iotas = _iota_arrays()
    in_maps = []
    for core in range(NCORES):
        b, r = divmod(core, 4)
        pol, half = divmod(r, 2)
        pv = 1.0 if pol == 0 else -1.0
        idx = np.nonzero(events[b, :, 3] == pv)[0]
        nh = (len(idx) + 1) // 2
        sl = idx[:nh] if half == 0 else idx[nh:]
        in_maps.append({
            "fields": _pack_fields(events[b][sl], flow[b][sl]),
            "iotas": iotas,
        })
    return in_maps


def kernel(events, flow):
    global LAST_EXEC_NS, LAST_RESULTS
    events = np.asarray(events, dtype=np.float32)
    B = events.shape[0]
    assert B == 2, events.shape

    nc = _get_prog()
    in_maps = make_in_maps(events, flow)
    res = run_bass_kernel_spmd(nc, in_maps, core_ids=list(range(NCORES)))
    LAST_RESULTS = res
    LAST_EXEC_NS = res.exec_time_ns

    hists = [[None, None], [None, None]]
    for b in range(2):
        for pol in range(2):
            hb = np.zeros((4, P, 512), np.float64)
            for half in range(2):
                hb += res.results[b * 4 + pol * 2 + half]["hist"]
            hists[b][pol] = hb
    return np.float32(loss_from_hists(hists))
